# revision 1
# baseline (speedup 1.0000x reference)
"""BiDAF-style co-attention (memory_len=2) Trainium2 Bass kernel.

Full inputs:
  input     [8, 4096, 1024] f32
  memory    [8, 2, 1024]    f32
  w_input   [1024] f32, w_memory [1024] f32, dot_scale [1024] f32
Output:
  concat([input, output_one, input*output_one, output_two*output_one], -1)
  -> [8, 4096, 4096] f32

Sharding: data-parallel over batch; core b gets batch b (8 cores).

Math (per batch):
  v_m   = w_input + dot_scale * mem_m            (d-vector, m=0,1)
  c_m   = mem_m . w_memory                       (scalar)
  att[l,m] = input[l] . v_m + c_m                (two fused mult-reduce DVE ops)
  amax[l] = max_m att[l,m]  (shifted by -c0; softmax over L is shift-inv)
  e_m[l] = exp(att[l,m]-amax[l]); r[l] = 1/(e0+e1); w1 = e1*r
  output_one[l] = r[l]*(e0[l]*mem0 + e1[l]*mem1)  (PE rank-2 outer product,
                                                   r applied on PSUM->SBUF copy)
  wexp[l] = exp(amax[l]); output_two = (sum_l wexp[l]*input[l]) / sum wexp
            (DVE in-place fused multiply-accumulate + one PE column reduce)
  comp3[l] = output_two*output_one = r*(e0*q0 + e1*q1), q_m = output_two*mem_m
            (even tiles: PE outer product; odd tiles: DVE fused q0 + w1*(q1-q0))
"""

import numpy as np

B, L, D = 8, 4096, 1024
T = L // 128  # 32 row-tiles of 128
G = 8         # tiles per group (batched small ops, dense PE bursts)

_CACHE = {}

# stats column layout ([128, NSTAT] f32), blocks of 32 (col t = tile t)
A0 = 0      # att0
A1 = 32     # att1
AM = 64     # amax
E0 = 96     # e0arg -> e0   (E1 = E0+32 so (e0_t, e1_t) is a stride-32 pair)
E1 = 128    # e1arg -> e1
WE = 160    # wexp = exp(amax)
SS = 192    # e0+e1
RR = 224    # r = 1/(e0+e1)
W1 = 256    # w1 = e1*r
OC = 288    # ones column
CD, SE, MD, C1, CF, ST, SR = 289, 290, 291, 292, 293, 294, 295
NSTAT = 296


def _build():
    import concourse.bacc as bacc
    import concourse.bass as bass
    import concourse.tile as tile
    from concourse import mybir
    from concourse.masks import make_identity

    f32 = mybir.dt.float32
    ALU = mybir.AluOpType
    ACT = mybir.ActivationFunctionType

    nc = bacc.Bacc("TRN2", target_bir_lowering=False, debug=False)

    inp = nc.dram_tensor("input", [L, D], f32, kind="ExternalInput").ap()
    mem = nc.dram_tensor("memory", [2, D], f32, kind="ExternalInput").ap()
    w_in = nc.dram_tensor("w_input", [D], f32, kind="ExternalInput").ap()
    w_mem = nc.dram_tensor("w_memory", [D], f32, kind="ExternalInput").ap()
    d_sc = nc.dram_tensor("dot_scale", [D], f32, kind="ExternalInput").ap()
    out = nc.dram_tensor("out", [L, 4 * D], f32, kind="ExternalOutput").ap()
    scr = nc.dram_tensor("scr", [8, D], f32).ap()  # internal scratch rows

    def bc(src_ap, n_part, n_free):
        # broadcast-read AP: n_part partitions each reading the same n_free
        # contiguous elements at src_ap's offset (DMA-only pattern)
        return bass.AP(src_ap.tensor, src_ap.offset, [[0, n_part], [1, n_free]])

    ts = bass.ts

    with tile.TileContext(nc) as tc:
        with (
            tc.tile_pool(name="consts", bufs=1) as consts,
            tc.tile_pool(name="setup2d", bufs=3) as setup2d,
            tc.tile_pool(name="setup1d", bufs=2) as setup1d,
            tc.tile_pool(name="bigbc", bufs=2) as bigbc,
            tc.tile_pool(name="inp_pool", bufs=16) as inp_pool,
            tc.tile_pool(name="scratch", bufs=2) as scratch,
            tc.tile_pool(name="stage12", bufs=5) as stage12p,
            tc.tile_pool(name="stage3", bufs=4) as stage3p,
            tc.tile_pool(name="o1ps", bufs=2, space="PSUM") as o1psp,
            tc.tile_pool(name="wstps", bufs=2, space="PSUM") as wstpsp,
        ):
            # ---------------- setup ----------------
            mem_sb = consts.tile([2, D], f32)
            nc.sync.dma_start(out=mem_sb, in_=mem)
            stats = consts.tile([128, NSTAT], f32)
            identity = consts.tile([128, 128], f32)
            make_identity(nc, identity)
            nc.vector.memset(stats[:, OC : OC + 1], 1.0)
            # est: per-tile transposed [e0;e1] stationaries, col-block t
            est = consts.tile([2, T * 128], f32)
            qcat = consts.tile([2, D], f32)
            o2p = consts.tile([128, D], f32)  # per-partition output_two partials
            nc.vector.memset(o2p, 0.0)
            # strided pair view: pair_view[:, t, :] = cols (E0+t, E1+t)
            pair_view = stats[:, E0 : E0 + 64].rearrange("p (a b) -> p b a", a=2)

            ds_b = setup2d.tile([2, D], f32, tag="s2d")
            nc.sync.dma_start(out=ds_b, in_=bc(d_sc, 2, D))
            win_b = setup2d.tile([2, D], f32, tag="s2d")
            nc.sync.dma_start(out=win_b, in_=bc(w_in, 2, D))
            # v_cat = mem*ds + w_in  (rows: v0, v1)
            vcat = setup2d.tile([2, D], f32, tag="s2d")
            nc.vector.tensor_tensor(out=vcat, in0=mem_sb, in1=ds_b, op=ALU.mult)
            nc.vector.tensor_tensor(out=vcat, in0=vcat, in1=win_b, op=ALU.add)
            nc.sync.dma_start(out=scr[0:2, :], in_=vcat)
            v0b = bigbc.tile([128, D], f32, tag="bb")
            nc.sync.dma_start(out=v0b, in_=bc(scr[0, :], 128, D))
            v1b = bigbc.tile([128, D], f32, tag="bb")
            nc.sync.dma_start(out=v1b, in_=bc(scr[1, :], 128, D))

            # memdot = (mem * w_memory).sum(-1) -> [2,1]; cdiff = c1-c0
            wmem_b = setup2d.tile([2, D], f32, tag="s2d")
            nc.sync.dma_start(out=wmem_b, in_=bc(w_mem, 2, D))
            sc2 = setup2d.tile([2, D], f32, tag="s2d")
            nc.vector.scalar_tensor_tensor(
                out=sc2, in0=mem_sb, scalar=1.0, in1=wmem_b,
                op0=ALU.mult, op1=ALU.mult,
                accum_out=stats[0:2, MD : MD + 1],
            )
            nc.sync.dma_start(out=scr[2, 0:2], in_=stats[0:2, MD : MD + 1])
            nc.sync.dma_start(out=stats[0:1, C1 : C1 + 1], in_=scr[2, 1:2])
            nc.vector.tensor_tensor(
                out=stats[0:1, CF : CF + 1],
                in0=stats[0:1, C1 : C1 + 1],
                in1=stats[0:1, MD : MD + 1],
                op=ALU.subtract,
            )
            nc.sync.dma_start(out=scr[2, 2:3], in_=stats[0:1, CF : CF + 1])
            nc.sync.dma_start(out=stats[:, CD : CD + 1], in_=bc(scr[2, 2:3], 128, 1))

            cdc = stats[:, CD : CD + 1]

            # ---------------- main pass ----------------
            for g in range(0, T, G):
                in_ts = {}
                # per-tile: load + two fused att dots (DVE)
                for t in range(g, g + G):
                    in_t = inp_pool.tile([128, D], f32, tag="in_t")
                    in_ts[t] = in_t
                    nc.sync.dma_start(out=in_t, in_=inp[ts(t, 128), :])
                    # comp0: passthrough copy of input
                    nc.gpsimd.dma_start(out=out[ts(t, 128), 0:D], in_=in_t)
                    sc_t = scratch.tile([128, D], f32, tag="ttr")
                    nc.vector.scalar_tensor_tensor(
                        out=sc_t, in0=in_t, scalar=1.0, in1=v0b,
                        op0=ALU.mult, op1=ALU.mult,
                        accum_out=stats[:, A0 + t : A0 + t + 1],
                    )
                    sc_t2 = scratch.tile([128, D], f32, tag="ttr")
                    nc.vector.scalar_tensor_tensor(
                        out=sc_t2, in0=in_t, scalar=1.0, in1=v1b,
                        op0=ALU.mult, op1=ALU.mult,
                        accum_out=stats[:, A1 + t : A1 + t + 1],
                    )

                # batched group stats ([128, G] blocks)
                a0b = stats[:, A0 + g : A0 + g + G]
                a1b = stats[:, A1 + g : A1 + g + G]
                amb = stats[:, AM + g : AM + g + G]
                e0b = stats[:, E0 + g : E0 + g + G]
                e1b = stats[:, E1 + g : E1 + g + G]
                web = stats[:, WE + g : WE + g + G]
                ssb = stats[:, SS + g : SS + g + G]
                rrb = stats[:, RR + g : RR + g + G]
                w1b = stats[:, W1 + g : W1 + g + G]
                # amax = max(a1 + cdiff, a0)
                nc.vector.scalar_tensor_tensor(
                    out=amb, in0=a1b, scalar=cdc, in1=a0b,
                    op0=ALU.add, op1=ALU.max,
                )
                # e0arg = a0 - amax ; e1arg = (a1 + cdiff) - amax
                nc.vector.tensor_tensor(out=e0b, in0=a0b, in1=amb, op=ALU.subtract)
                nc.vector.scalar_tensor_tensor(
                    out=e1b, in0=a1b, scalar=cdc, in1=amb,
                    op0=ALU.add, op1=ALU.subtract,
                )
                nc.scalar.activation(out=e0b, in_=e0b, func=ACT.Exp)
                nc.scalar.activation(out=e1b, in_=e1b, func=ACT.Exp)
                nc.scalar.activation(out=web, in_=amb, func=ACT.Exp)
                nc.vector.tensor_tensor(out=ssb, in0=e0b, in1=e1b, op=ALU.add)
                nc.vector.reciprocal(rrb, ssb)
                nc.vector.tensor_tensor(out=w1b, in0=e1b, in1=rrb, op=ALU.mult)

                # per-tile: PE outer product + comp2 + output_two accum
                for t in range(g, g + G):
                    in_t = in_ts[t]
                    rc = stats[:, RR + t : RR + t + 1]
                    # stationary [2,128] = transpose of the (e0_t, e1_t) pair
                    wst_ps = wstpsp.tile([2, 128], f32, tag="wst")
                    nc.tensor.transpose(wst_ps, pair_view[:, t, :], identity)
                    nc.scalar.copy(est[:, ts(t, 128)], wst_ps)

                    # output_one numerator = e0*mem0 + e1*mem1
                    o1_ps = o1psp.tile([128, D], f32, tag="o1")
                    for h in range(2):
                        nc.tensor.matmul(
                            o1_ps[:, ts(h, 512)],
                            lhsT=est[:, ts(t, 128)],
                            rhs=mem_sb[:, ts(h, 512)],
                            start=True,
                            stop=True,
                        )
                    st12 = stage12p.tile([128, 2 * D], f32, tag="s12")
                    # normalized output_one on the PSUM->SBUF copy
                    nc.scalar.activation(
                        out=st12[:, 0:D], in_=o1_ps, func=ACT.Copy, scale=rc
                    )
                    # comp2 = input * output_one = (input*r) * o1_num
                    nc.vector.scalar_tensor_tensor(
                        out=st12[:, D : 2 * D], in0=in_t, scalar=rc, in1=o1_ps,
                        op0=ALU.mult, op1=ALU.mult,
                    )
                    nc.scalar.dma_start(out=out[ts(t, 128), D : 3 * D], in_=st12)
                    # output_two partials: o2p += wexp_t * in_t  (in-place DVE)
                    nc.vector.scalar_tensor_tensor(
                        out=o2p, in0=in_t,
                        scalar=stats[:, WE + t : WE + t + 1], in1=o2p,
                        op0=ALU.mult, op1=ALU.add,
                    )

            # ---------------- output_two normalize + q vectors ----------------
            # column-sum of o2p via PE (ones stationary)
            o2s_ps = o1psp.tile([128, D], f32, tag="o1")
            for h in range(2):
                nc.tensor.matmul(
                    o2s_ps[0:1, ts(h, 512)],
                    lhsT=stats[:, OC : OC + 1],
                    rhs=o2p[:, ts(h, 512)],
                    start=True,
                    stop=True,
                )
            nc.vector.tensor_reduce(
                out=stats[:, SE : SE + 1], in_=stats[:, WE : WE + T],
                axis=mybir.AxisListType.X, op=ALU.add,
            )
            stot_ps = wstpsp.tile([1, 1], f32, tag="wst")
            nc.tensor.matmul(
                stot_ps, lhsT=stats[:, SE : SE + 1], rhs=stats[:, OC : OC + 1],
                start=True, stop=True,
            )
            nc.scalar.copy(stats[0:1, ST : ST + 1], stot_ps)
            nc.vector.reciprocal(stats[0:1, SR : SR + 1], stats[0:1, ST : ST + 1])

            o2_sb = setup1d.tile([1, D], f32, tag="s1d")
            nc.scalar.copy(o2_sb, o2s_ps[0:1, :])
            o2n = setup1d.tile([1, D], f32, tag="s1d")
            nc.vector.tensor_scalar_mul(o2n, o2_sb, stats[0:1, SR : SR + 1])
            nc.sync.dma_start(out=scr[4, :], in_=o2n)
            o2ncat = setup2d.tile([2, D], f32, tag="s2d")
            nc.sync.dma_start(out=o2ncat, in_=bc(scr[4, :], 2, D))
            nc.vector.tensor_tensor(out=qcat, in0=o2ncat, in1=mem_sb, op=ALU.mult)
            # q0 / qdiff broadcasts for the DVE comp3 tiles
            nc.sync.dma_start(out=scr[5:7, :], in_=qcat)
            q0_b = bigbc.tile([128, D], f32, tag="bb")
            nc.sync.dma_start(out=q0_b, in_=bc(scr[5, :], 128, D))
            q1_p0 = setup1d.tile([1, D], f32, tag="s1d")
            nc.sync.dma_start(out=q1_p0, in_=scr[6, :])
            qd_p0 = setup1d.tile([1, D], f32, tag="s1d")
            nc.vector.tensor_tensor(
                out=qd_p0, in0=q1_p0, in1=qcat[0:1, :], op=ALU.subtract
            )
            nc.sync.dma_start(out=scr[7, :], in_=qd_p0)
            qd_b = bigbc.tile([128, D], f32, tag="bb")
            nc.sync.dma_start(out=qd_b, in_=bc(scr[7, :], 128, D))

            # ---------------- comp3 pass (DVE fused) --------------------------
            for t in range(T):
                st3 = stage3p.tile([128, D], f32, tag="s3")
                nc.vector.scalar_tensor_tensor(
                    out=st3, in0=qd_b,
                    scalar=stats[:, W1 + t : W1 + t + 1],
                    in1=q0_b, op0=ALU.mult, op1=ALU.add,
                )
                nc.gpsimd.dma_start(out=out[ts(t, 128), 3 * D : 4 * D], in_=st3)

    nc.compile()
    return nc


def _get_nc():
    if "nc" not in _CACHE:
        _CACHE["nc"] = _build()
    return _CACHE["nc"]


def kernel(input, memory, w_input, w_memory, dot_scale):
    from concourse.bass_utils import run_bass_kernel_spmd

    nc = _get_nc()
    input = np.ascontiguousarray(input, dtype=np.float32)
    memory = np.ascontiguousarray(memory, dtype=np.float32)
    w_input = np.ascontiguousarray(w_input, dtype=np.float32)
    w_memory = np.ascontiguousarray(w_memory, dtype=np.float32)
    dot_scale = np.ascontiguousarray(dot_scale, dtype=np.float32)
    in_maps = [
        {
            "input": input[b],
            "memory": memory[b],
            "w_input": w_input,
            "w_memory": w_memory,
            "dot_scale": dot_scale,
        }
        for b in range(B)
    ]
    res = run_bass_kernel_spmd(nc, in_maps, core_ids=list(range(B)))
    return np.stack([res.results[b]["out"] for b in range(B)], axis=0)



# revision 13
# speedup vs baseline: 1.0320x; 1.0320x over previous
"""BiDAF-style co-attention (memory_len=2) Trainium2 Bass kernel.

Full inputs:
  input     [8, 4096, 1024] f32
  memory    [8, 2, 1024]    f32
  w_input   [1024] f32, w_memory [1024] f32, dot_scale [1024] f32
Output:
  concat([input, output_one, input*output_one, output_two*output_one], -1)
  -> [8, 4096, 4096] f32

Sharding: data-parallel over batch; core b gets batch b (8 cores).

Math (per batch):
  v_m   = w_input + dot_scale * mem_m            (d-vector, m=0,1)
  c_m   = mem_m . w_memory                       (scalar)
  att[l,m] = input[l] . v_m + c_m                (two fused mult-reduce DVE ops)
  amax[l] = max_m att[l,m]  (shifted by -c0; softmax over L is shift-inv)
  e_m[l] = exp(att[l,m]-amax[l]); r[l] = 1/(e0+e1)
  output_one[l] = r[l]*(e0[l]*mem0 + e1[l]*mem1)  (PE rank-2 outer product)
  wexp[l] = exp(amax[l]); output_two = (sum_l wexp[l]*input[l]) / sum wexp
            (PE column-reduce matmuls accumulated in PSUM, f32r)
  comp3[l] = output_two*output_one = r*(e0*q0 + e1*q1), q_m = output_two*mem_m
            (even tiles: PE outer product; odd tiles: DVE fused q0 + w1*(q1-q0))

Schedule: groups of 4 row-tiles. Per group one 2MB input read (sync ring),
one 2MB comp0 write (sync ring, straight from the input tiles), one 4MB
comp1|comp2 write (scalar ring). comp3 written in a second pass (global
softmax dependency), alternating rings. All DMA is HWDGE; no SWDGE
(gpsimd descriptor generation stalls behind DVE port locks). All
broadcasts/reductions stay on-chip via PE outer products.
"""

import numpy as np

B, L, D = 8, 4096, 1024
T = L // 128  # 32 row-tiles of 128
G = 4         # tiles per group (2MB input reads, 4MB st12 writes)
NG = T // G   # 8 groups

_CACHE = {}

# stats column layout ([128, NSTAT] f32), blocks of 32 (col t = tile t)
A0 = 0      # att0 (dot with v0)
A1 = 32     # att1 (dot with v1, without cdiff)
AM = 64     # amax (shifted by -c0)
E0 = 96     # e0arg -> e0   (E1 = E0+32 so (e0_t, e1_t) is a stride-32 pair)
E1 = 128    # e1arg -> e1
WE = 160    # wexp = exp(amax)
SS = 192    # e0+e1
RR = 224    # r = 1/(e0+e1)
W1 = 256    # w1 = e1*r (comp3 odd tiles)
CD, SE, C01, CF, ST, SR = 288, 289, 290, 292, 293, 294
NSTAT = 296


def _build():
    import concourse.bacc as bacc
    import concourse.bass as bass
    import concourse.tile as tile
    from concourse import mybir
    from concourse.masks import make_identity

    f32 = mybir.dt.float32
    bf16 = mybir.dt.bfloat16
    ALU = mybir.AluOpType
    ACT = mybir.ActivationFunctionType

    nc = bacc.Bacc("TRN2", target_bir_lowering=False, debug=False)

    inp = nc.dram_tensor("input", [L, D], f32, kind="ExternalInput").ap()
    mem = nc.dram_tensor("memory", [2, D], f32, kind="ExternalInput").ap()
    w_in = nc.dram_tensor("w_input", [D], f32, kind="ExternalInput").ap()
    w_mem = nc.dram_tensor("w_memory", [D], f32, kind="ExternalInput").ap()
    d_sc = nc.dram_tensor("dot_scale", [D], f32, kind="ExternalInput").ap()
    out = nc.dram_tensor("out", [L, 4 * D], f32, kind="ExternalOutput").ap()

    def bc(src_ap, n_part, n_free):
        # broadcast-read AP: n_part partitions each reading the same n_free
        # contiguous elements at src_ap's offset (DMA-only pattern)
        return bass.AP(src_ap.tensor, src_ap.offset, [[0, n_part], [1, n_free]])

    ts = bass.ts

    def grp_rows(g):
        return slice(g * G * 128, (g + 1) * G * 128)

    with tile.TileContext(nc) as tc:
        with (
            tc.tile_pool(name="consts", bufs=1) as consts,
            tc.tile_pool(name="setup2d", bufs=4) as setup2d,
            tc.tile_pool(name="inp_pool", bufs=2) as inp_pool,
            tc.tile_pool(name="scratch", bufs=2) as scratch,
            tc.tile_pool(name="st12", bufs=2) as st12p,
            tc.tile_pool(name="inbf", bufs=2) as inbfp,
            tc.tile_pool(name="st3", bufs=2) as st3p,
            tc.tile_pool(name="o1ps", bufs=2, space="PSUM") as o1psp,
            tc.tile_pool(name="smps", bufs=2, space="PSUM") as smpsp,
            tc.tile_pool(name="o2ps", bufs=1, space="PSUM") as o2psp,
        ):
            # ---------------- persistent tiles ----------------
            stats = consts.tile([128, NSTAT], f32)
            identity = consts.tile([128, 128], f32)
            make_identity(nc, identity)
            ones_r = consts.tile([1, 128], f32)   # ones row: broadcast lhsT
            nc.vector.memset(ones_r, 1.0)
            ones_c = consts.tile([128, 1], f32)   # ones col: column reduce
            nc.vector.memset(ones_c, 1.0)
            # row-select stationaries: sel[:, m*128:(m+1)*128].T @ x = bcast x[m]
            # (engine APs must start at partition 0: build with nested memsets)
            sel = consts.tile([2, 256], f32)
            nc.vector.memset(sel, 0.0)
            nc.vector.memset(sel[0:1, 0:128], 1.0)
            nc.vector.memset(sel[0:2, 128:256], 1.0)
            nc.vector.memset(sel[0:1, 128:256], 0.0)
            # pmB.T @ x = broadcast of (x[1] - x[0])
            pmB = consts.tile([2, 128], f32)
            nc.vector.memset(pmB, 1.0)
            nc.vector.memset(pmB[0:1, :], -1.0)
            mem_sb = consts.tile([2, D], f32)
            mem_bf = consts.tile([2, D], bf16)    # bf16 copy for PE
            est = consts.tile([2, T * 128], bf16)  # per-tile (e0;e1).T stationaries
            weB = consts.tile([128, T], bf16)     # wexp in bf16 for PE o2p
            vb = consts.tile([128, 2 * D], f32)   # v0 | v1 broadcast rows
            o2n = consts.tile([1, D], f32)        # normalized output_two
            qcat = consts.tile([2, D], f32)       # q_m = o2n * mem_m
            qcat_bf = consts.tile([2, D], bf16)   # bf16 copy for PE
            q0b = consts.tile([128, D], bf16)     # q0 broadcast
            qdb = consts.tile([128, D], bf16)     # q1-q0 broadcast
            # strided pair view: pair_view[:, t, :] = cols (E0+t, E1+t)
            pair_view = stats[:, E0 : E0 + 64].rearrange("p (a b) -> p b a", a=2)
            o2_ps = o2psp.tile([1, D], f32, tag="o2")  # held across main pass

            # ---------------- issue first reads, then setup ----------------
            in_grps = {}
            in_grps[0] = inp_pool.tile([128, G, D], f32, tag="in_g", name="in_g0")
            nc.sync.dma_start(
                out=in_grps[0],
                in_=inp[grp_rows(0), :].rearrange("(i p) c -> p i c", p=128),
            )
            # small loads on the scalar ring so the sync ring starts with R0
            nc.scalar.dma_start(out=mem_sb, in_=mem)
            ds_b = setup2d.tile([2, D], f32, tag="s2d")
            nc.scalar.dma_start(out=ds_b, in_=bc(d_sc, 2, D))
            win_b = setup2d.tile([2, D], f32, tag="s2d")
            nc.scalar.dma_start(out=win_b, in_=bc(w_in, 2, D))
            wmem_b = setup2d.tile([2, D], f32, tag="s2d")
            nc.scalar.dma_start(out=wmem_b, in_=bc(w_mem, 2, D))
            in_grps[1] = inp_pool.tile([128, G, D], f32, tag="in_g", name="in_g1")
            nc.sync.dma_start(
                out=in_grps[1],
                in_=inp[grp_rows(1), :].rearrange("(i p) c -> p i c", p=128),
            )

            # v_cat = mem*ds + w_in  (rows: v0, v1)
            vcat = setup2d.tile([2, D], f32, tag="s2d")
            nc.vector.tensor_tensor(out=vcat, in0=mem_sb, in1=ds_b, op=ALU.mult)
            nc.vector.tensor_tensor(out=vcat, in0=vcat, in1=win_b, op=ALU.add)
            # broadcast v0/v1 to 128 partitions via PE outer product (fp32)
            for m in range(2):
                for h in range(2):
                    bp = smpsp.tile([128, 512], f32, tag="sm")
                    nc.tensor.matmul(
                        bp, lhsT=sel[:, ts(m, 128)], rhs=vcat[:, ts(h, 512)],
                        start=True, stop=True,
                    )
                    nc.scalar.copy(vb[:, ts(2 * m + h, 512)], bp)

            nc.scalar.copy(mem_bf, mem_sb)

            # memdot = (mem * w_memory).sum(-1); cdiff = c1-c0 -> bcast col CD
            sc2 = setup2d.tile([2, D], f32, tag="s2d")
            nc.vector.scalar_tensor_tensor(
                out=sc2, in0=mem_sb, scalar=1.0, in1=wmem_b,
                op0=ALU.mult, op1=ALU.mult,
                accum_out=stats[0:2, CD : CD + 1],
            )
            mdt = smpsp.tile([128, 512], f32, tag="sm")
            nc.tensor.transpose(
                mdt[0:1, 0:2], stats[0:2, CD : CD + 1], identity[0:2, 0:2]
            )
            nc.scalar.copy(stats[0:1, C01 : C01 + 2], mdt[0:1, 0:2])
            nc.vector.tensor_tensor(
                out=stats[0:1, CF : CF + 1],
                in0=stats[0:1, C01 + 1 : C01 + 2],
                in1=stats[0:1, C01 : C01 + 1],
                op=ALU.subtract,
            )
            cdp = smpsp.tile([128, 512], f32, tag="sm")
            nc.tensor.matmul(
                cdp[:, 0:1], lhsT=ones_r, rhs=stats[0:1, CF : CF + 1],
                start=True, stop=True,
            )
            nc.scalar.copy(stats[:, CD : CD + 1], cdp[:, 0:1])
            cdc = stats[:, CD : CD + 1]

            # ---------------- main pass ----------------
            for g in range(NG):
                in_g = in_grps.pop(g)
                # comp0 write (straight copy of the loaded tiles) must be
                # issued BEFORE the prefetch read on the same sync ring:
                # the prefetch reuses this pool slot two groups later and
                # a FIFO ring cannot reorder around the WAR dependency.
                nc.sync.dma_start(
                    out=out[grp_rows(g), 0:D].rearrange("(i p) c -> p i c", p=128),
                    in_=in_g,
                )
                if g + 2 < NG:
                    nxt = inp_pool.tile([128, G, D], f32, tag="in_g", name="in_gn")
                    in_grps[g + 2] = nxt
                    nc.sync.dma_start(
                        out=nxt,
                        in_=inp[grp_rows(g + 2), :].rearrange(
                            "(i p) c -> p i c", p=128
                        ),
                    )

                # att dots (DVE, fused mult-reduce)
                for i in range(G):
                    t = g * G + i
                    in_t = in_g[:, i, :]
                    sc_t = scratch.tile([128, D], bf16, tag="ttr")
                    nc.vector.scalar_tensor_tensor(
                        out=sc_t, in0=in_t, scalar=1.0, in1=vb[:, 0:D],
                        op0=ALU.mult, op1=ALU.mult,
                        accum_out=stats[:, A0 + t : A0 + t + 1],
                    )
                    sc_t2 = scratch.tile([128, D], bf16, tag="ttr")
                    nc.vector.scalar_tensor_tensor(
                        out=sc_t2, in0=in_t, scalar=1.0, in1=vb[:, D : 2 * D],
                        op0=ALU.mult, op1=ALU.mult,
                        accum_out=stats[:, A1 + t : A1 + t + 1],
                    )

                # batched group stats ([128, G] blocks)
                a0b = stats[:, A0 + g * G : A0 + g * G + G]
                a1b = stats[:, A1 + g * G : A1 + g * G + G]
                amb = stats[:, AM + g * G : AM + g * G + G]
                e0b = stats[:, E0 + g * G : E0 + g * G + G]
                e1b = stats[:, E1 + g * G : E1 + g * G + G]
                ssb = stats[:, SS + g * G : SS + g * G + G]
                rrb = stats[:, RR + g * G : RR + g * G + G]
                # amax = max(a1 + cdiff, a0)
                nc.vector.scalar_tensor_tensor(
                    out=amb, in0=a1b, scalar=cdc, in1=a0b,
                    op0=ALU.add, op1=ALU.max,
                )
                # e0arg = a0 - amax ; e1arg = (a1 + cdiff) - amax
                nc.vector.tensor_tensor(out=e0b, in0=a0b, in1=amb, op=ALU.subtract)
                nc.vector.scalar_tensor_tensor(
                    out=e1b, in0=a1b, scalar=cdc, in1=amb,
                    op0=ALU.add, op1=ALU.subtract,
                )
                nc.scalar.activation(out=e0b, in_=e0b, func=ACT.Exp)
                nc.scalar.activation(out=e1b, in_=e1b, func=ACT.Exp)
                nc.scalar.activation(
                    out=weB[:, g * G : g * G + G], in_=amb, func=ACT.Exp
                )
                nc.vector.tensor_tensor(out=ssb, in0=e0b, in1=e1b, op=ALU.add)
                nc.vector.reciprocal(rrb, ssb)

                # per-tile transposed stationaries (e0_t, e1_t) -> est
                in_bf = inbfp.tile([128, G, D], bf16, tag="inbf")
                for i in range(G):
                    t = g * G + i
                    wst = smpsp.tile([128, 512], f32, tag="sm")
                    nc.tensor.transpose(
                        wst[0:2, 0:128], pair_view[:, t, :], identity
                    )
                    nc.scalar.copy(est[:, ts(t, 128)], wst[0:2, 0:128])
                    nc.scalar.copy(in_bf[:, i, :], in_g[:, i, :])

                st12 = st12p.tile([128, G, 2 * D], f32, tag="s12")
                for i in range(G):
                    t = g * G + i
                    in_t = in_g[:, i, :]
                    rc = stats[:, RR + t : RR + t + 1]
                    # output_one numerator = e0*mem0 + e1*mem1 (PE rank-2)
                    o1_ps = o1psp.tile([128, D], f32, tag="o1")
                    for h in range(2):
                        nc.tensor.matmul(
                            o1_ps[:, ts(h, 512)],
                            lhsT=est[:, ts(t, 128)],
                            rhs=mem_bf[:, ts(h, 512)],
                            start=True,
                            stop=True,
                        )
                    # output_two partials: o2_ps += wexp_t^T @ in_bf_t
                    for h in range(2):
                        nc.tensor.matmul(
                            o2_ps[0:1, ts(h, 512)],
                            lhsT=weB[:, t : t + 1],
                            rhs=in_bf[:, i, ts(h, 512)],
                            start=(t == 0),
                            stop=(t == T - 1),
                            skip_group_check=True,
                        )
                    # comp1 = r * o1_num on the PSUM->SBUF copy
                    nc.scalar.activation(
                        out=st12[:, i, 0:D], in_=o1_ps, func=ACT.Copy, scale=rc
                    )
                    # comp2 = input * output_one = (input*r) * o1_num
                    nc.vector.scalar_tensor_tensor(
                        out=st12[:, i, D : 2 * D], in0=in_t, scalar=rc,
                        in1=o1_ps, op0=ALU.mult, op1=ALU.mult,
                    )

                # group write: comp1|comp2 on the scalar ring
                nc.scalar.dma_start(
                    out=out[grp_rows(g), D : 3 * D].rearrange(
                        "(i p) c -> p i c", p=128
                    ),
                    in_=st12,
                )

            # ---------------- output_two normalize + q vectors ----------------
            # w1 = e1*r for the DVE comp3 tiles (single batched op)
            nc.vector.tensor_tensor(
                out=stats[:, W1 : W1 + T],
                in0=stats[:, E1 : E1 + T],
                in1=stats[:, RR : RR + T],
                op=ALU.mult,
            )
            # total wexp sum: row-reduce WE block, column-reduce via PE
            nc.vector.tensor_reduce(
                out=stats[:, SE : SE + 1], in_=weB,
                axis=mybir.AxisListType.X, op=ALU.add,
            )
            stp = smpsp.tile([128, 512], f32, tag="sm")
            nc.tensor.matmul(
                stp[0:1, 0:1], lhsT=stats[:, SE : SE + 1], rhs=ones_c,
                start=True, stop=True,
            )
            nc.scalar.copy(stats[0:1, ST : ST + 1], stp[0:1, 0:1])
            nc.vector.reciprocal(stats[0:1, SR : SR + 1], stats[0:1, ST : ST + 1])
            # o2n = o2_ps / total
            nc.scalar.activation(
                out=o2n, in_=o2_ps[0:1, :], func=ACT.Copy,
                scale=stats[0:1, SR : SR + 1],
            )
            # qcat = (o2n broadcast to 2 rows) * mem
            qps = o1psp.tile([128, D], f32, tag="o1")
            for h in range(2):
                nc.tensor.matmul(
                    qps[0:2, ts(h, 512)], lhsT=ones_r[0:1, 0:2],
                    rhs=o2n[0:1, ts(h, 512)], start=True, stop=True,
                )
            nc.vector.tensor_tensor(
                out=qcat, in0=qps[0:2, :], in1=mem_sb, op=ALU.mult
            )
            nc.scalar.copy(qcat_bf, qcat)
            # q0 / qdiff broadcasts for the DVE comp3 tiles
            for h in range(2):
                bp = smpsp.tile([128, 512], f32, tag="sm")
                nc.tensor.matmul(
                    bp, lhsT=sel[:, 0:128], rhs=qcat[:, ts(h, 512)],
                    start=True, stop=True,
                )
                nc.scalar.copy(q0b[:, ts(h, 512)], bp)
                bp2 = smpsp.tile([128, 512], f32, tag="sm")
                nc.tensor.matmul(
                    bp2, lhsT=pmB, rhs=qcat[:, ts(h, 512)],
                    start=True, stop=True,
                )
                nc.scalar.copy(qdb[:, ts(h, 512)], bp2)

            # ---------------- comp3 pass --------------------------
            # even tiles: PE outer product est_t.T @ qcat (r on copy-out)
            # odd tiles: DVE fused q0 + w1*(q1-q0)
            for g in range(NG):
                st3 = st3p.tile([128, G, D], f32, tag="s3")
                for i in range(G):
                    t = g * G + i
                    if i % 2 == 0:
                        c3ps = o1psp.tile([128, D], f32, tag="o1")
                        for h in range(2):
                            nc.tensor.matmul(
                                c3ps[:, ts(h, 512)],
                                lhsT=est[:, ts(t, 128)],
                                rhs=qcat_bf[:, ts(h, 512)],
                                start=True,
                                stop=True,
                            )
                        nc.scalar.activation(
                            out=st3[:, i, :], in_=c3ps, func=ACT.Copy,
                            scale=stats[:, RR + t : RR + t + 1],
                        )
                    else:
                        nc.vector.scalar_tensor_tensor(
                            out=st3[:, i, :], in0=qdb,
                            scalar=stats[:, W1 + t : W1 + t + 1],
                            in1=q0b, op0=ALU.mult, op1=ALU.add,
                        )
                eng = nc.sync if g % 2 == 0 else nc.scalar
                eng.dma_start(
                    out=out[grp_rows(g), 3 * D : 4 * D].rearrange(
                        "(i p) c -> p i c", p=128
                    ),
                    in_=st3,
                )

    nc.compile()
    return nc


def _get_nc():
    if "nc" not in _CACHE:
        _CACHE["nc"] = _build()
    return _CACHE["nc"]


def kernel(input, memory, w_input, w_memory, dot_scale):
    from concourse.bass_utils import run_bass_kernel_spmd

    nc = _get_nc()
    input = np.ascontiguousarray(input, dtype=np.float32)
    memory = np.ascontiguousarray(memory, dtype=np.float32)
    w_input = np.ascontiguousarray(w_input, dtype=np.float32)
    w_memory = np.ascontiguousarray(w_memory, dtype=np.float32)
    dot_scale = np.ascontiguousarray(dot_scale, dtype=np.float32)
    in_maps = [
        {
            "input": input[b],
            "memory": memory[b],
            "w_input": w_input,
            "w_memory": w_memory,
            "dot_scale": dot_scale,
        }
        for b in range(B)
    ]
    res = run_bass_kernel_spmd(nc, in_maps, core_ids=list(range(B)))
    return np.stack([res.results[b]["out"] for b in range(B)], axis=0)


# revision 14
# speedup vs baseline: 1.1328x; 1.0977x over previous
"""BiDAF-style co-attention (memory_len=2) Trainium2 Bass kernel.

Full inputs:
  input     [8, 4096, 1024] f32
  memory    [8, 2, 1024]    f32
  w_input   [1024] f32, w_memory [1024] f32, dot_scale [1024] f32
Output:
  concat([input, output_one, input*output_one, output_two*output_one], -1)
  -> [8, 4096, 4096] f32

Sharding: data-parallel over batch; core b gets batch b (8 cores).

Math (per batch):
  v_m   = w_input + dot_scale * mem_m            (d-vector, m=0,1)
  c_m   = mem_m . w_memory                       (scalar)
  att[l,m] = input[l] . v_m + c_m                (two fused mult-reduce DVE ops)
  amax[l] = max_m att[l,m]  (shifted by -c0; softmax over L is shift-inv)
  e_m[l] = exp(att[l,m]-amax[l]); r[l] = 1/(e0+e1)
  output_one[l] = r[l]*(e0[l]*mem0 + e1[l]*mem1)  (PE rank-2 outer product)
  wexp[l] = exp(amax[l]); output_two = (sum_l wexp[l]*input[l]) / sum wexp
            (PE column-reduce matmuls accumulated in PSUM, f32r)
  comp3[l] = output_two*output_one = r*(e0*q0 + e1*q1), q_m = output_two*mem_m
            (even tiles: PE outer product; odd tiles: DVE fused q0 + w1*(q1-q0))

Schedule: groups of 4 row-tiles. Per group one 2MB input read (sync ring),
one 2MB comp0 write (sync ring, straight from the input tiles), one 4MB
comp1|comp2 write (scalar ring). comp3 written in a second pass (global
softmax dependency), alternating rings. All DMA is HWDGE; no SWDGE
(gpsimd descriptor generation stalls behind DVE port locks). All
broadcasts/reductions stay on-chip via PE outer products.
"""

import numpy as np

B, L, D = 8, 4096, 1024
T = L // 128  # 32 row-tiles of 128
G = 4         # tiles per group (2MB input reads, 4MB st12 writes)
NG = T // G   # 8 groups

_CACHE = {}

# stats column layout ([128, NSTAT] f32), blocks of 32 (col t = tile t)
A0 = 0      # att0 (dot with v0)
A1 = 32     # att1 (dot with v1, without cdiff)
AM = 64     # amax (shifted by -c0)
E0 = 96     # e0arg -> e0   (E1 = E0+32 so (e0_t, e1_t) is a stride-32 pair)
E1 = 128    # e1arg -> e1
WE = 160    # wexp = exp(amax)
SS = 192    # e0+e1
RR = 224    # r = 1/(e0+e1)
W1 = 256    # w1 = e1*r (comp3 odd tiles)
CD, SE, C01, CF, ST, SR = 288, 289, 290, 292, 293, 294
NSTAT = 296


def _build():
    import concourse.bacc as bacc
    import concourse.bass as bass
    import concourse.tile as tile
    from concourse import mybir
    from concourse.masks import make_identity

    f32 = mybir.dt.float32
    bf16 = mybir.dt.bfloat16
    ALU = mybir.AluOpType
    ACT = mybir.ActivationFunctionType

    nc = bacc.Bacc("TRN2", target_bir_lowering=False, debug=False)

    inp = nc.dram_tensor("input", [L, D], f32, kind="ExternalInput").ap()
    mem = nc.dram_tensor("memory", [2, D], f32, kind="ExternalInput").ap()
    w_in = nc.dram_tensor("w_input", [D], f32, kind="ExternalInput").ap()
    w_mem = nc.dram_tensor("w_memory", [D], f32, kind="ExternalInput").ap()
    d_sc = nc.dram_tensor("dot_scale", [D], f32, kind="ExternalInput").ap()
    out = nc.dram_tensor("out", [L, 4 * D], f32, kind="ExternalOutput").ap()

    def bc(src_ap, n_part, n_free):
        # broadcast-read AP: n_part partitions each reading the same n_free
        # contiguous elements at src_ap's offset (DMA-only pattern)
        return bass.AP(src_ap.tensor, src_ap.offset, [[0, n_part], [1, n_free]])

    ts = bass.ts

    def grp_rows(g):
        return slice(g * G * 128, (g + 1) * G * 128)

    with tile.TileContext(nc) as tc:
        with (
            tc.tile_pool(name="consts", bufs=1) as consts,
            tc.tile_pool(name="setup2d", bufs=4) as setup2d,
            tc.tile_pool(name="inp_pool", bufs=2) as inp_pool,
            tc.tile_pool(name="scratch", bufs=2) as scratch,
            tc.tile_pool(name="st12", bufs=2) as st12p,
            tc.tile_pool(name="inbf", bufs=2) as inbfp,
            tc.tile_pool(name="st3", bufs=2) as st3p,
            tc.tile_pool(name="o1ps", bufs=2, space="PSUM") as o1psp,
            tc.tile_pool(name="smps", bufs=2, space="PSUM") as smpsp,
            tc.tile_pool(name="o2ps", bufs=1, space="PSUM") as o2psp,
        ):
            # ---------------- persistent tiles ----------------
            stats = consts.tile([128, NSTAT], f32)
            identity = consts.tile([128, 128], f32)
            make_identity(nc, identity)
            ones_r = consts.tile([1, 128], f32)   # ones row: broadcast lhsT
            nc.vector.memset(ones_r, 1.0)
            ones_c = consts.tile([128, 1], f32)   # ones col: column reduce
            nc.vector.memset(ones_c, 1.0)
            # row-select stationaries: sel[:, m*128:(m+1)*128].T @ x = bcast x[m]
            # (engine APs must start at partition 0: build with nested memsets)
            sel = consts.tile([2, 256], f32)
            nc.vector.memset(sel, 0.0)
            nc.vector.memset(sel[0:1, 0:128], 1.0)
            nc.vector.memset(sel[0:2, 128:256], 1.0)
            nc.vector.memset(sel[0:1, 128:256], 0.0)
            # pmB.T @ x = broadcast of (x[1] - x[0])
            pmB = consts.tile([2, 128], f32)
            nc.vector.memset(pmB, 1.0)
            nc.vector.memset(pmB[0:1, :], -1.0)
            mem_sb = consts.tile([2, D], f32)
            mem_bf = consts.tile([2, D], bf16)    # bf16 copy for PE
            est = consts.tile([2, T * 128], bf16)  # per-tile (e0;e1).T stationaries
            weB = consts.tile([128, T], bf16)     # wexp in bf16 for PE o2p
            vb = consts.tile([128, 2 * D], f32)   # v0 | v1 broadcast rows
            o2n = consts.tile([1, D], f32)        # normalized output_two
            qcat = consts.tile([2, D], f32)       # q_m = o2n * mem_m
            qcat_bf = consts.tile([2, D], bf16)   # bf16 copy for PE
            q0b = consts.tile([128, D], bf16)     # q0 broadcast
            qdb = consts.tile([128, D], bf16)     # q1-q0 broadcast
            # strided pair view: pair_view[:, t, :] = cols (E0+t, E1+t)
            pair_view = stats[:, E0 : E0 + 64].rearrange("p (a b) -> p b a", a=2)
            o2_ps = o2psp.tile([1, D], f32, tag="o2")  # held across main pass

            # ---------------- issue first reads, then setup ----------------
            in_grps = {}
            in_grps[0] = inp_pool.tile([128, G, D], f32, tag="in_g", name="in_g0")
            nc.sync.dma_start(
                out=in_grps[0],
                in_=inp[grp_rows(0), :].rearrange("(i p) c -> p i c", p=128),
            )
            # small loads on the scalar ring so the sync ring starts with R0
            nc.scalar.dma_start(out=mem_sb, in_=mem)
            ds_b = setup2d.tile([2, D], f32, tag="s2d")
            nc.scalar.dma_start(out=ds_b, in_=bc(d_sc, 2, D))
            win_b = setup2d.tile([2, D], f32, tag="s2d")
            nc.scalar.dma_start(out=win_b, in_=bc(w_in, 2, D))
            wmem_b = setup2d.tile([2, D], f32, tag="s2d")
            nc.scalar.dma_start(out=wmem_b, in_=bc(w_mem, 2, D))
            in_grps[1] = inp_pool.tile([128, G, D], f32, tag="in_g", name="in_g1")
            nc.sync.dma_start(
                out=in_grps[1],
                in_=inp[grp_rows(1), :].rearrange("(i p) c -> p i c", p=128),
            )

            # v_cat = mem*ds + w_in  (rows: v0, v1)
            vcat = setup2d.tile([2, D], f32, tag="s2d")
            nc.vector.tensor_tensor(out=vcat, in0=mem_sb, in1=ds_b, op=ALU.mult)
            nc.vector.tensor_tensor(out=vcat, in0=vcat, in1=win_b, op=ALU.add)
            # broadcast v0/v1 to 128 partitions via PE outer product (fp32)
            for m in range(2):
                for h in range(2):
                    bp = smpsp.tile([128, 512], f32, tag="sm")
                    nc.tensor.matmul(
                        bp, lhsT=sel[:, ts(m, 128)], rhs=vcat[:, ts(h, 512)],
                        start=True, stop=True,
                    )
                    nc.scalar.copy(vb[:, ts(2 * m + h, 512)], bp)

            nc.scalar.copy(mem_bf, mem_sb)

            # memdot = (mem * w_memory).sum(-1); cdiff = c1-c0 -> bcast col CD
            sc2 = setup2d.tile([2, D], f32, tag="s2d")
            nc.vector.scalar_tensor_tensor(
                out=sc2, in0=mem_sb, scalar=1.0, in1=wmem_b,
                op0=ALU.mult, op1=ALU.mult,
                accum_out=stats[0:2, CD : CD + 1],
            )
            mdt = smpsp.tile([128, 512], f32, tag="sm")
            nc.tensor.transpose(
                mdt[0:1, 0:2], stats[0:2, CD : CD + 1], identity[0:2, 0:2]
            )
            nc.scalar.copy(stats[0:1, C01 : C01 + 2], mdt[0:1, 0:2])
            nc.vector.tensor_tensor(
                out=stats[0:1, CF : CF + 1],
                in0=stats[0:1, C01 + 1 : C01 + 2],
                in1=stats[0:1, C01 : C01 + 1],
                op=ALU.subtract,
            )
            cdp = smpsp.tile([128, 512], f32, tag="sm")
            nc.tensor.matmul(
                cdp[:, 0:1], lhsT=ones_r, rhs=stats[0:1, CF : CF + 1],
                start=True, stop=True,
            )
            nc.scalar.copy(stats[:, CD : CD + 1], cdp[:, 0:1])
            cdc = stats[:, CD : CD + 1]

            # ---------------- main pass (2-stage software pipeline) --------
            # period p: stage A computes att/stats/stationaries for group p
            # while stage B runs matmuls/comp1/comp2/writes for group p-1.
            # Every cross-engine dependency edge (exp -> transpose -> est copy
            # -> matmul -> comp1/comp2) gets a full period of slack, so no
            # engine queue ever blocks on a chain issued the same period.
            def stage_a(g):
                in_g = in_grps[g]
                # comp0 write: plain copy of the loaded tiles (sync ring)
                nc.sync.dma_start(
                    out=out[grp_rows(g), 0:D].rearrange("(i p) c -> p i c", p=128),
                    in_=in_g,
                )
                # att dots (DVE, fused mult-reduce)
                for i in range(G):
                    t = g * G + i
                    in_t = in_g[:, i, :]
                    sc_t = scratch.tile([128, D], bf16, tag="ttr")
                    nc.vector.scalar_tensor_tensor(
                        out=sc_t, in0=in_t, scalar=1.0, in1=vb[:, 0:D],
                        op0=ALU.mult, op1=ALU.mult,
                        accum_out=stats[:, A0 + t : A0 + t + 1],
                    )
                    sc_t2 = scratch.tile([128, D], bf16, tag="ttr")
                    nc.vector.scalar_tensor_tensor(
                        out=sc_t2, in0=in_t, scalar=1.0, in1=vb[:, D : 2 * D],
                        op0=ALU.mult, op1=ALU.mult,
                        accum_out=stats[:, A1 + t : A1 + t + 1],
                    )
                # batched group stats ([128, G] blocks)
                a0b = stats[:, A0 + g * G : A0 + g * G + G]
                a1b = stats[:, A1 + g * G : A1 + g * G + G]
                amb = stats[:, AM + g * G : AM + g * G + G]
                e0b = stats[:, E0 + g * G : E0 + g * G + G]
                e1b = stats[:, E1 + g * G : E1 + g * G + G]
                ssb = stats[:, SS + g * G : SS + g * G + G]
                rrb = stats[:, RR + g * G : RR + g * G + G]
                nc.vector.scalar_tensor_tensor(
                    out=amb, in0=a1b, scalar=cdc, in1=a0b,
                    op0=ALU.add, op1=ALU.max,
                )
                nc.vector.tensor_tensor(out=e0b, in0=a0b, in1=amb, op=ALU.subtract)
                nc.vector.scalar_tensor_tensor(
                    out=e1b, in0=a1b, scalar=cdc, in1=amb,
                    op0=ALU.add, op1=ALU.subtract,
                )
                nc.scalar.activation(out=e0b, in_=e0b, func=ACT.Exp)
                nc.scalar.activation(out=e1b, in_=e1b, func=ACT.Exp)
                nc.scalar.activation(
                    out=weB[:, g * G : g * G + G], in_=amb, func=ACT.Exp
                )
                # ssb/rrb issued on DVE now but only consumed next period
                nc.vector.tensor_tensor(out=ssb, in0=e0b, in1=e1b, op=ALU.add)
                nc.vector.reciprocal(rrb, ssb)
                # per-tile transposed stationaries (e0_t, e1_t) -> est (bf16)
                for i in range(G):
                    t = g * G + i
                    wst = smpsp.tile([128, 512], f32, tag="sm")
                    nc.tensor.transpose(
                        wst[0:2, 0:128], pair_view[:, t, :], identity
                    )
                    nc.scalar.copy(est[:, ts(t, 128)], wst[0:2, 0:128])

            def stage_b(g):
                in_g = in_grps.pop(g)
                in_bf = inbfp.tile([128, G, D], bf16, tag="inbf")
                for i in range(G):
                    nc.scalar.copy(in_bf[:, i, :], in_g[:, i, :])
                st12 = st12p.tile([128, G, 2 * D], f32, tag="s12")
                for i in range(G):
                    t = g * G + i
                    in_t = in_g[:, i, :]
                    rc = stats[:, RR + t : RR + t + 1]
                    # output_one numerator = e0*mem0 + e1*mem1 (PE rank-2)
                    o1_ps = o1psp.tile([128, D], f32, tag="o1")
                    for h in range(2):
                        nc.tensor.matmul(
                            o1_ps[:, ts(h, 512)],
                            lhsT=est[:, ts(t, 128)],
                            rhs=mem_bf[:, ts(h, 512)],
                            start=True,
                            stop=True,
                        )
                    # output_two partials: o2_ps += wexp_t^T @ in_bf_t
                    for h in range(2):
                        nc.tensor.matmul(
                            o2_ps[0:1, ts(h, 512)],
                            lhsT=weB[:, t : t + 1],
                            rhs=in_bf[:, i, ts(h, 512)],
                            start=(t == 0),
                            stop=(t == T - 1),
                            skip_group_check=True,
                        )
                    # comp1 = r * o1_num on the PSUM->SBUF copy
                    nc.scalar.activation(
                        out=st12[:, i, 0:D], in_=o1_ps, func=ACT.Copy, scale=rc
                    )
                    # comp2 = input * output_one = (input*r) * o1_num
                    nc.vector.scalar_tensor_tensor(
                        out=st12[:, i, D : 2 * D], in0=in_t, scalar=rc,
                        in1=o1_ps, op0=ALU.mult, op1=ALU.mult,
                    )
                # comp1|comp2 group write on the scalar ring
                nc.scalar.dma_start(
                    out=out[grp_rows(g), D : 3 * D].rearrange(
                        "(i p) c -> p i c", p=128
                    ),
                    in_=st12,
                )
                # prefetch read g+2: issued after this stage's in_g readers,
                # because it reuses the same pool slot (FIFO ring WAR order)
                if g + 2 < NG:
                    nxt = inp_pool.tile([128, G, D], f32, tag="in_g", name="in_gn")
                    in_grps[g + 2] = nxt
                    nc.sync.dma_start(
                        out=nxt,
                        in_=inp[grp_rows(g + 2), :].rearrange(
                            "(i p) c -> p i c", p=128
                        ),
                    )

            for p in range(NG + 1):
                if p < NG:
                    stage_a(p)
                if p >= 1:
                    stage_b(p - 1)

            # ---------------- output_two normalize + q vectors ----------------
            # w1 = e1*r for the DVE comp3 tiles (single batched op)
            nc.vector.tensor_tensor(
                out=stats[:, W1 : W1 + T],
                in0=stats[:, E1 : E1 + T],
                in1=stats[:, RR : RR + T],
                op=ALU.mult,
            )
            # total wexp sum: row-reduce WE block, column-reduce via PE
            nc.vector.tensor_reduce(
                out=stats[:, SE : SE + 1], in_=weB,
                axis=mybir.AxisListType.X, op=ALU.add,
            )
            stp = smpsp.tile([128, 512], f32, tag="sm")
            nc.tensor.matmul(
                stp[0:1, 0:1], lhsT=stats[:, SE : SE + 1], rhs=ones_c,
                start=True, stop=True,
            )
            nc.scalar.copy(stats[0:1, ST : ST + 1], stp[0:1, 0:1])
            nc.vector.reciprocal(stats[0:1, SR : SR + 1], stats[0:1, ST : ST + 1])
            # o2n = o2_ps / total
            nc.scalar.activation(
                out=o2n, in_=o2_ps[0:1, :], func=ACT.Copy,
                scale=stats[0:1, SR : SR + 1],
            )
            # qcat = (o2n broadcast to 2 rows) * mem
            qps = o1psp.tile([128, D], f32, tag="o1")
            for h in range(2):
                nc.tensor.matmul(
                    qps[0:2, ts(h, 512)], lhsT=ones_r[0:1, 0:2],
                    rhs=o2n[0:1, ts(h, 512)], start=True, stop=True,
                )
            nc.vector.tensor_tensor(
                out=qcat, in0=qps[0:2, :], in1=mem_sb, op=ALU.mult
            )
            nc.scalar.copy(qcat_bf, qcat)
            # q0 / qdiff broadcasts for the DVE comp3 tiles
            for h in range(2):
                bp = smpsp.tile([128, 512], f32, tag="sm")
                nc.tensor.matmul(
                    bp, lhsT=sel[:, 0:128], rhs=qcat[:, ts(h, 512)],
                    start=True, stop=True,
                )
                nc.scalar.copy(q0b[:, ts(h, 512)], bp)
                bp2 = smpsp.tile([128, 512], f32, tag="sm")
                nc.tensor.matmul(
                    bp2, lhsT=pmB, rhs=qcat[:, ts(h, 512)],
                    start=True, stop=True,
                )
                nc.scalar.copy(qdb[:, ts(h, 512)], bp2)

            # ---------------- comp3 pass --------------------------
            # even tiles: PE outer product est_t.T @ qcat (r on copy-out)
            # odd tiles: DVE fused q0 + w1*(q1-q0)
            for g in range(NG):
                st3 = st3p.tile([128, G, D], f32, tag="s3")
                for i in range(G):
                    t = g * G + i
                    if i % 2 == 0:
                        c3ps = o1psp.tile([128, D], f32, tag="o1")
                        for h in range(2):
                            nc.tensor.matmul(
                                c3ps[:, ts(h, 512)],
                                lhsT=est[:, ts(t, 128)],
                                rhs=qcat_bf[:, ts(h, 512)],
                                start=True,
                                stop=True,
                            )
                        nc.scalar.activation(
                            out=st3[:, i, :], in_=c3ps, func=ACT.Copy,
                            scale=stats[:, RR + t : RR + t + 1],
                        )
                    else:
                        nc.vector.scalar_tensor_tensor(
                            out=st3[:, i, :], in0=qdb,
                            scalar=stats[:, W1 + t : W1 + t + 1],
                            in1=q0b, op0=ALU.mult, op1=ALU.add,
                        )
                eng = nc.sync if g % 2 == 0 else nc.scalar
                eng.dma_start(
                    out=out[grp_rows(g), 3 * D : 4 * D].rearrange(
                        "(i p) c -> p i c", p=128
                    ),
                    in_=st3,
                )

    nc.compile()
    return nc


def _get_nc():
    if "nc" not in _CACHE:
        _CACHE["nc"] = _build()
    return _CACHE["nc"]


def kernel(input, memory, w_input, w_memory, dot_scale):
    from concourse.bass_utils import run_bass_kernel_spmd

    nc = _get_nc()
    input = np.ascontiguousarray(input, dtype=np.float32)
    memory = np.ascontiguousarray(memory, dtype=np.float32)
    w_input = np.ascontiguousarray(w_input, dtype=np.float32)
    w_memory = np.ascontiguousarray(w_memory, dtype=np.float32)
    dot_scale = np.ascontiguousarray(dot_scale, dtype=np.float32)
    in_maps = [
        {
            "input": input[b],
            "memory": memory[b],
            "w_input": w_input,
            "w_memory": w_memory,
            "dot_scale": dot_scale,
        }
        for b in range(B)
    ]
    res = run_bass_kernel_spmd(nc, in_maps, core_ids=list(range(B)))
    return np.stack([res.results[b]["out"] for b in range(B)], axis=0)


# revision 15
# speedup vs baseline: 1.2884x; 1.1373x over previous
"""BiDAF-style co-attention (memory_len=2) Trainium2 Bass kernel.

Full inputs:
  input     [8, 4096, 1024] f32
  memory    [8, 2, 1024]    f32
  w_input   [1024] f32, w_memory [1024] f32, dot_scale [1024] f32
Output:
  concat([input, output_one, input*output_one, output_two*output_one], -1)
  -> [8, 4096, 4096] f32

Sharding: data-parallel over batch; core b gets batch b (8 cores).

Math (per batch):
  v_m   = w_input + dot_scale * mem_m            (d-vector, m=0,1)
  c_m   = mem_m . w_memory                       (scalar)
  att[l,m] = input[l] . v_m + c_m                (two fused mult-reduce DVE ops)
  amax[l] = max_m att[l,m]  (shifted by -c0; softmax over L is shift-inv)
  e_m[l] = exp(att[l,m]-amax[l]); r[l] = 1/(e0+e1)
  output_one[l] = r[l]*(e0[l]*mem0 + e1[l]*mem1)  (PE rank-2 outer product)
  wexp[l] = exp(amax[l]); output_two = (sum_l wexp[l]*input[l]) / sum wexp
            (PE column-reduce matmuls accumulated in PSUM, f32r)
  comp3[l] = output_two*output_one = r*(e0*q0 + e1*q1), q_m = output_two*mem_m
            (even tiles: PE outer product; odd tiles: DVE fused q0 + w1*(q1-q0))

Schedule: groups of 4 row-tiles. Per group one 2MB input read (sync ring),
one 2MB comp0 write (sync ring, straight from the input tiles), one 4MB
comp1|comp2 write (scalar ring). comp3 written in a second pass (global
softmax dependency), alternating rings. All DMA is HWDGE; no SWDGE
(gpsimd descriptor generation stalls behind DVE port locks). All
broadcasts/reductions stay on-chip via PE outer products.
"""

import numpy as np

B, L, D = 8, 4096, 1024
T = L // 128  # 32 row-tiles of 128
G = 4         # tiles per group (2MB input reads, 4MB st12 writes)
NG = T // G   # 8 groups

_CACHE = {}

# stats column layout ([128, NSTAT] f32), blocks of 32 (col t = tile t)
A0 = 0      # att0 (dot with v0)
A1 = 32     # att1 (dot with v1, without cdiff)
AM = 64     # amax (shifted by -c0)
E0 = 96     # e0arg -> e0   (E1 = E0+32 so (e0_t, e1_t) is a stride-32 pair)
E1 = 128    # e1arg -> e1
WE = 160    # wexp = exp(amax)
SS = 192    # e0+e1
RR = 224    # r = 1/(e0+e1)
W1 = 256    # w1 = e1*r (comp3 odd tiles)
CD, SE, C01, CF, ST, SR = 288, 289, 290, 292, 293, 294
NSTAT = 296


def _build():
    import concourse.bacc as bacc
    import concourse.bass as bass
    import concourse.tile as tile
    from concourse import mybir
    from concourse.masks import make_identity

    f32 = mybir.dt.float32
    bf16 = mybir.dt.bfloat16
    ALU = mybir.AluOpType
    ACT = mybir.ActivationFunctionType

    nc = bacc.Bacc("TRN2", target_bir_lowering=False, debug=False)

    inp = nc.dram_tensor("input", [L, D], f32, kind="ExternalInput").ap()
    mem = nc.dram_tensor("memory", [2, D], f32, kind="ExternalInput").ap()
    w_in = nc.dram_tensor("w_input", [D], f32, kind="ExternalInput").ap()
    w_mem = nc.dram_tensor("w_memory", [D], f32, kind="ExternalInput").ap()
    d_sc = nc.dram_tensor("dot_scale", [D], f32, kind="ExternalInput").ap()
    out = nc.dram_tensor("out", [L, 4 * D], f32, kind="ExternalOutput").ap()

    def bc(src_ap, n_part, n_free):
        # broadcast-read AP: n_part partitions each reading the same n_free
        # contiguous elements at src_ap's offset (DMA-only pattern)
        return bass.AP(src_ap.tensor, src_ap.offset, [[0, n_part], [1, n_free]])

    ts = bass.ts

    def grp_rows(g):
        return slice(g * G * 128, (g + 1) * G * 128)

    with tile.TileContext(nc) as tc:
        with (
            tc.tile_pool(name="consts", bufs=1) as consts,
            tc.tile_pool(name="setup2d", bufs=4) as setup2d,
            tc.tile_pool(name="inp_pool", bufs=4) as inp_pool,
            tc.tile_pool(name="scratch", bufs=2) as scratch,
            tc.tile_pool(name="st12", bufs=2) as st12p,
            tc.tile_pool(name="inbf", bufs=2) as inbfp,
            tc.tile_pool(name="o1ps", bufs=2, space="PSUM") as o1psp,
            tc.tile_pool(name="smps", bufs=2, space="PSUM") as smpsp,
            tc.tile_pool(name="o2ps", bufs=1, space="PSUM") as o2psp,
        ):
            # ---------------- persistent tiles ----------------
            stats = consts.tile([128, NSTAT], f32)
            identity = consts.tile([128, 128], f32)
            ones_r = consts.tile([1, 128], f32)   # ones row: broadcast lhsT
            ones_c = consts.tile([128, 1], f32)   # ones col: column reduce
            sel = consts.tile([2, 256], f32)
            pmB = consts.tile([2, 128], f32)
            mem_sb = consts.tile([2, D], f32)
            mem_bf = consts.tile([2, D], bf16)    # bf16 copy for PE
            est = consts.tile([2, T * 128], bf16)  # per-tile (e0;e1).T stationaries
            weB = consts.tile([128, T], bf16)     # wexp in bf16 for PE o2p
            vb = consts.tile([128, 2 * D], f32)   # v0 | v1 broadcast rows
            o2n = consts.tile([1, D], f32)        # normalized output_two
            qcat = consts.tile([2, D], f32)       # q_m = o2n * mem_m
            qcat_bf = consts.tile([2, D], bf16)   # bf16 copy for PE
            q0b = consts.tile([128, D], bf16)     # q0 broadcast
            qdb = consts.tile([128, D], bf16)     # q1-q0 broadcast
            # strided pair view: pair_view[:, t, :] = cols (E0+t, E1+t)
            pair_view = stats[:, E0 : E0 + 64].rearrange("p (a b) -> p b a", a=2)
            o2_ps = o2psp.tile([1, D], f32, tag="o2")  # held across main pass

            # ---------------- issue first reads, then setup ----------------
            in_grps = {}
            in_grps[0] = inp_pool.tile([128, G, D], f32, tag="in_g", name="in_g0")
            nc.sync.dma_start(
                out=in_grps[0],
                in_=inp[grp_rows(0), :].rearrange("(i p) c -> p i c", p=128),
            )
            # small loads on the scalar ring so the sync ring starts with R0
            nc.scalar.dma_start(out=mem_sb, in_=mem)
            ds_b = setup2d.tile([2, D], f32, tag="s2d")
            nc.scalar.dma_start(out=ds_b, in_=bc(d_sc, 2, D))
            win_b = setup2d.tile([2, D], f32, tag="s2d")
            nc.scalar.dma_start(out=win_b, in_=bc(w_in, 2, D))
            wmem_b = setup2d.tile([2, D], f32, tag="s2d")
            nc.scalar.dma_start(out=wmem_b, in_=bc(w_mem, 2, D))
            in_grps[1] = inp_pool.tile([128, G, D], f32, tag="in_g", name="in_g1")
            nc.sync.dma_start(
                out=in_grps[1],
                in_=inp[grp_rows(1), :].rearrange("(i p) c -> p i c", p=128),
            )
            # prefetch groups 2,3 immediately (bufs=4): the ring never waits
            in_grps[2] = inp_pool.tile([128, G, D], f32, tag="in_g", name="in_g2")
            nc.sync.dma_start(
                out=in_grps[2],
                in_=inp[grp_rows(2), :].rearrange("(i p) c -> p i c", p=128),
            )
            in_grps[3] = inp_pool.tile([128, G, D], f32, tag="in_g", name="in_g3")
            nc.sync.dma_start(
                out=in_grps[3],
                in_=inp[grp_rows(3), :].rearrange("(i p) c -> p i c", p=128),
            )
            make_identity(nc, identity)
            nc.vector.memset(ones_r, 1.0)
            nc.vector.memset(ones_c, 1.0)
            # row-select stationaries: sel[:, m*128:(m+1)*128].T @ x = bcast x[m]
            # (engine APs must start at partition 0: build with nested memsets)
            nc.vector.memset(sel, 0.0)
            nc.vector.memset(sel[0:1, 0:128], 1.0)
            nc.vector.memset(sel[0:2, 128:256], 1.0)
            nc.vector.memset(sel[0:1, 128:256], 0.0)
            # pmB.T @ x = broadcast of (x[1] - x[0])
            nc.vector.memset(pmB, 1.0)
            nc.vector.memset(pmB[0:1, :], -1.0)

            # v_cat = mem*ds + w_in  (rows: v0, v1)
            vcat = setup2d.tile([2, D], f32, tag="s2d")
            nc.vector.tensor_tensor(out=vcat, in0=mem_sb, in1=ds_b, op=ALU.mult)
            nc.vector.tensor_tensor(out=vcat, in0=vcat, in1=win_b, op=ALU.add)
            # broadcast v0/v1 to 128 partitions via PE outer product (fp32)
            for m in range(2):
                for h in range(2):
                    bp = smpsp.tile([128, 512], f32, tag="sm")
                    nc.tensor.matmul(
                        bp, lhsT=sel[:, ts(m, 128)], rhs=vcat[:, ts(h, 512)],
                        start=True, stop=True,
                    )
                    nc.scalar.copy(vb[:, ts(2 * m + h, 512)], bp)

            nc.scalar.copy(mem_bf, mem_sb)

            # memdot = (mem * w_memory).sum(-1); cdiff = c1-c0 -> bcast col CD
            sc2 = setup2d.tile([2, D], f32, tag="s2d")
            nc.vector.scalar_tensor_tensor(
                out=sc2, in0=mem_sb, scalar=1.0, in1=wmem_b,
                op0=ALU.mult, op1=ALU.mult,
                accum_out=stats[0:2, CD : CD + 1],
            )
            mdt = smpsp.tile([128, 512], f32, tag="sm")
            nc.tensor.transpose(
                mdt[0:1, 0:2], stats[0:2, CD : CD + 1], identity[0:2, 0:2]
            )
            nc.scalar.copy(stats[0:1, C01 : C01 + 2], mdt[0:1, 0:2])
            nc.vector.tensor_tensor(
                out=stats[0:1, CF : CF + 1],
                in0=stats[0:1, C01 + 1 : C01 + 2],
                in1=stats[0:1, C01 : C01 + 1],
                op=ALU.subtract,
            )
            cdp = smpsp.tile([128, 512], f32, tag="sm")
            nc.tensor.matmul(
                cdp[:, 0:1], lhsT=ones_r, rhs=stats[0:1, CF : CF + 1],
                start=True, stop=True,
            )
            nc.scalar.copy(stats[:, CD : CD + 1], cdp[:, 0:1])
            cdc = stats[:, CD : CD + 1]

            # ---------------- main pass (2-stage software pipeline) --------
            # period p: stage A computes att/stats/stationaries for group p
            # while stage B runs matmuls/comp1/comp2/writes for group p-1.
            # Every cross-engine dependency edge (exp -> transpose -> est copy
            # -> matmul -> comp1/comp2) gets a full period of slack, so no
            # engine queue ever blocks on a chain issued the same period.
            def stage_a(g):
                in_g = in_grps[g]
                # comp0 write: plain copy of the loaded tiles (sync ring)
                nc.sync.dma_start(
                    out=out[grp_rows(g), 0:D].rearrange("(i p) c -> p i c", p=128),
                    in_=in_g,
                )
                # att dots (DVE, fused mult-reduce)
                for i in range(G):
                    t = g * G + i
                    in_t = in_g[:, i, :]
                    sc_t = scratch.tile([128, D], bf16, tag="ttr")
                    nc.vector.scalar_tensor_tensor(
                        out=sc_t, in0=in_t, scalar=1.0, in1=vb[:, 0:D],
                        op0=ALU.mult, op1=ALU.mult,
                        accum_out=stats[:, A0 + t : A0 + t + 1],
                    )
                    sc_t2 = scratch.tile([128, D], bf16, tag="ttr")
                    nc.vector.scalar_tensor_tensor(
                        out=sc_t2, in0=in_t, scalar=1.0, in1=vb[:, D : 2 * D],
                        op0=ALU.mult, op1=ALU.mult,
                        accum_out=stats[:, A1 + t : A1 + t + 1],
                    )
                # batched group stats ([128, G] blocks)
                a0b = stats[:, A0 + g * G : A0 + g * G + G]
                a1b = stats[:, A1 + g * G : A1 + g * G + G]
                amb = stats[:, AM + g * G : AM + g * G + G]
                e0b = stats[:, E0 + g * G : E0 + g * G + G]
                e1b = stats[:, E1 + g * G : E1 + g * G + G]
                ssb = stats[:, SS + g * G : SS + g * G + G]
                rrb = stats[:, RR + g * G : RR + g * G + G]
                nc.vector.scalar_tensor_tensor(
                    out=amb, in0=a1b, scalar=cdc, in1=a0b,
                    op0=ALU.add, op1=ALU.max,
                )
                nc.vector.tensor_tensor(out=e0b, in0=a0b, in1=amb, op=ALU.subtract)
                nc.vector.scalar_tensor_tensor(
                    out=e1b, in0=a1b, scalar=cdc, in1=amb,
                    op0=ALU.add, op1=ALU.subtract,
                )
                nc.scalar.activation(out=e0b, in_=e0b, func=ACT.Exp)
                nc.scalar.activation(out=e1b, in_=e1b, func=ACT.Exp)
                nc.scalar.activation(
                    out=weB[:, g * G : g * G + G], in_=amb, func=ACT.Exp
                )
                # ssb/rrb issued on DVE now but only consumed next period
                nc.vector.tensor_tensor(out=ssb, in0=e0b, in1=e1b, op=ALU.add)
                nc.vector.reciprocal(rrb, ssb)
                # per-tile transposed stationaries (e0_t, e1_t) -> est (bf16)
                for i in range(G):
                    t = g * G + i
                    wst = smpsp.tile([128, 512], f32, tag="sm")
                    nc.tensor.transpose(
                        wst[0:2, 0:128], pair_view[:, t, :], identity
                    )
                    nc.scalar.copy(est[:, ts(t, 128)], wst[0:2, 0:128])

            def stage_b(g):
                in_g = in_grps.pop(g)
                in_bf = inbfp.tile([128, G, D], bf16, tag="inbf")
                for i in range(G):
                    nc.scalar.copy(in_bf[:, i, :], in_g[:, i, :])
                st12 = st12p.tile([128, G, 2 * D], f32, tag="s12")
                for i in range(G):
                    t = g * G + i
                    in_t = in_g[:, i, :]
                    rc = stats[:, RR + t : RR + t + 1]
                    # output_one numerator = e0*mem0 + e1*mem1 (PE rank-2)
                    o1_ps = o1psp.tile([128, D], f32, tag="o1")
                    for h in range(2):
                        nc.tensor.matmul(
                            o1_ps[:, ts(h, 512)],
                            lhsT=est[:, ts(t, 128)],
                            rhs=mem_bf[:, ts(h, 512)],
                            start=True,
                            stop=True,
                        )
                    # output_two partials: o2_ps += wexp_t^T @ in_bf_t
                    for h in range(2):
                        nc.tensor.matmul(
                            o2_ps[0:1, ts(h, 512)],
                            lhsT=weB[:, t : t + 1],
                            rhs=in_bf[:, i, ts(h, 512)],
                            start=(t == 0),
                            stop=(t == T - 1),
                            skip_group_check=True,
                        )
                    # comp1 = r * o1_num on the PSUM->SBUF copy
                    nc.scalar.activation(
                        out=st12[:, i, 0:D], in_=o1_ps, func=ACT.Copy, scale=rc
                    )
                    # comp2 = input * output_one = (input*r) * o1_num
                    nc.vector.scalar_tensor_tensor(
                        out=st12[:, i, D : 2 * D], in0=in_t, scalar=rc,
                        in1=o1_ps, op0=ALU.mult, op1=ALU.mult,
                    )
                # comp1|comp2 group write on the scalar ring
                nc.scalar.dma_start(
                    out=out[grp_rows(g), D : 3 * D].rearrange(
                        "(i p) c -> p i c", p=128
                    ),
                    in_=st12,
                )
                # prefetch read g+4: issued after this stage's in_g readers,
                # because it reuses the same pool slot (FIFO ring WAR order)
                if g + 4 < NG:
                    nxt = inp_pool.tile([128, G, D], f32, tag="in_g", name="in_gn")
                    in_grps[g + 4] = nxt
                    nc.sync.dma_start(
                        out=nxt,
                        in_=inp[grp_rows(g + 4), :].rearrange(
                            "(i p) c -> p i c", p=128
                        ),
                    )

            for p in range(NG + 1):
                if p < NG:
                    stage_a(p)
                if p >= 1:
                    stage_b(p - 1)

            # ---------------- output_two normalize + q vectors ----------------
            # w1 = e1*r for the DVE comp3 tiles (single batched op)
            nc.vector.tensor_tensor(
                out=stats[:, W1 : W1 + T],
                in0=stats[:, E1 : E1 + T],
                in1=stats[:, RR : RR + T],
                op=ALU.mult,
            )
            # total wexp sum: row-reduce WE block, column-reduce via PE
            nc.vector.tensor_reduce(
                out=stats[:, SE : SE + 1], in_=weB,
                axis=mybir.AxisListType.X, op=ALU.add,
            )
            stp = smpsp.tile([128, 512], f32, tag="sm")
            nc.tensor.matmul(
                stp[0:1, 0:1], lhsT=stats[:, SE : SE + 1], rhs=ones_c,
                start=True, stop=True,
            )
            nc.scalar.copy(stats[0:1, ST : ST + 1], stp[0:1, 0:1])
            nc.vector.reciprocal(stats[0:1, SR : SR + 1], stats[0:1, ST : ST + 1])
            # o2n = o2_ps / total
            nc.scalar.activation(
                out=o2n, in_=o2_ps[0:1, :], func=ACT.Copy,
                scale=stats[0:1, SR : SR + 1],
            )
            # qcat = (o2n broadcast to 2 rows) * mem
            qps = o1psp.tile([128, D], f32, tag="o1")
            for h in range(2):
                nc.tensor.matmul(
                    qps[0:2, ts(h, 512)], lhsT=ones_r[0:1, 0:2],
                    rhs=o2n[0:1, ts(h, 512)], start=True, stop=True,
                )
            nc.vector.tensor_tensor(
                out=qcat, in0=qps[0:2, :], in1=mem_sb, op=ALU.mult
            )
            nc.scalar.copy(qcat_bf, qcat)
            # q0 / qdiff broadcasts for the DVE comp3 tiles
            for h in range(2):
                bp = smpsp.tile([128, 512], f32, tag="sm")
                nc.tensor.matmul(
                    bp, lhsT=sel[:, 0:128], rhs=qcat[:, ts(h, 512)],
                    start=True, stop=True,
                )
                nc.scalar.copy(q0b[:, ts(h, 512)], bp)
                bp2 = smpsp.tile([128, 512], f32, tag="sm")
                nc.tensor.matmul(
                    bp2, lhsT=pmB, rhs=qcat[:, ts(h, 512)],
                    start=True, stop=True,
                )
                nc.scalar.copy(qdb[:, ts(h, 512)], bp2)

            # ---------------- comp3 pass --------------------------
            # even tiles: PE outer product est_t.T @ qcat (r on copy-out)
            # odd tiles: DVE fused q0 + w1*(q1-q0)
            for g in range(NG):
                st3 = inp_pool.tile([128, G, D], f32, tag="in_g", name="st3")
                for i in range(G):
                    t = g * G + i
                    if i % 2 == 0:
                        c3ps = o1psp.tile([128, D], f32, tag="o1")
                        for h in range(2):
                            nc.tensor.matmul(
                                c3ps[:, ts(h, 512)],
                                lhsT=est[:, ts(t, 128)],
                                rhs=qcat_bf[:, ts(h, 512)],
                                start=True,
                                stop=True,
                            )
                        nc.scalar.activation(
                            out=st3[:, i, :], in_=c3ps, func=ACT.Copy,
                            scale=stats[:, RR + t : RR + t + 1],
                        )
                    else:
                        nc.vector.scalar_tensor_tensor(
                            out=st3[:, i, :], in0=qdb,
                            scalar=stats[:, W1 + t : W1 + t + 1],
                            in1=q0b, op0=ALU.mult, op1=ALU.add,
                        )
                eng = nc.sync if g % 2 == 0 else nc.scalar
                eng.dma_start(
                    out=out[grp_rows(g), 3 * D : 4 * D].rearrange(
                        "(i p) c -> p i c", p=128
                    ),
                    in_=st3,
                )

    nc.compile()
    return nc


def _get_nc():
    if "nc" not in _CACHE:
        _CACHE["nc"] = _build()
    return _CACHE["nc"]


def kernel(input, memory, w_input, w_memory, dot_scale):
    from concourse.bass_utils import run_bass_kernel_spmd

    nc = _get_nc()
    input = np.ascontiguousarray(input, dtype=np.float32)
    memory = np.ascontiguousarray(memory, dtype=np.float32)
    w_input = np.ascontiguousarray(w_input, dtype=np.float32)
    w_memory = np.ascontiguousarray(w_memory, dtype=np.float32)
    dot_scale = np.ascontiguousarray(dot_scale, dtype=np.float32)
    in_maps = [
        {
            "input": input[b],
            "memory": memory[b],
            "w_input": w_input,
            "w_memory": w_memory,
            "dot_scale": dot_scale,
        }
        for b in range(B)
    ]
    res = run_bass_kernel_spmd(nc, in_maps, core_ids=list(range(B)))
    return np.stack([res.results[b]["out"] for b in range(B)], axis=0)
